# revision 6
# baseline (speedup 1.0000x reference)
"""Trainium2 Bass kernel for nn_CapsuleNet_3D (8 NeuronCores, SPMD).

Strategy:
  - Data-parallel over batch (16) for conv stem / primary caps / routing:
    2 batch elements per core.
  - conv1 (1->128, k=3, valid) as one K=28 matmul group over host-built
    im2col patches (27 patch rows + 1 ones row folding the bias).
  - primary caps conv (128->128, k=9, s=2) as 729 accumulated K=128
    matmuls in PSUM, with host-transposed weights [o, cin, cout] streamed
    through SBUF.
  - squash / routing priors / 3 dynamic-routing iterations on DVE/ACT with
    r=16000 laid out as [128 partitions x 125].
  - decoder fc1/fc2 replicated, fc3/fc4 column-split across the 8 cores
    (1280 / 3375 columns each); d-vector and fc3 activations exchanged with
    AllGather collectives; fc4 weights (138MB/core) streamed from HBM.
Outputs: classes [16,2], recon [16,27000].
"""

import numpy as np

import concourse.bass as bass
import concourse.bacc as bacc
import concourse.mybir as mybir
from concourse import tile
from concourse.bass_utils import run_bass_kernel_spmd

f32 = mybir.dt.float32
AF = mybir.ActivationFunctionType
ALU = mybir.AluOpType

N_CORES = 8
B = 16
BPC = 2              # batch per core
S1 = 27 * 27 * 27    # conv1 output spatial (19683)
K1 = 28              # 27 kernel taps + bias row
OFF = 729            # pcaps kernel taps (9^3)
SP = 1000            # pcaps output spatial (10^3)
DIM, SHI, SLO, CAP, O, C = 16, 8, 125, 8, 16, 2
D1, D2, D3, D4 = 512, 2048, 10240, 27000
D3PC, D4PC = D3 // N_CORES, D4 // N_CORES   # 1280, 3375
KT4 = D3 // 128      # 80 k-tiles for fc4
WT_CHUNK = 27        # pcaps weight taps per streamed chunk (27 | 729)
EPS = 1e-8

_CACHE = {}


def _squash_scale(nc, pool, n2, tag):
    """Given n2 = |t|^2 [P, F] (SBUF), return scale = n2/(1+n2)/(sqrt(n2)+eps)."""
    P, F = n2.shape
    n = pool.tile([P, F], f32, tag=tag + "_n")
    nc.scalar.activation(n[:], n2[:], AF.Sqrt)
    t1 = pool.tile([P, F], f32, tag=tag + "_t1")
    nc.vector.tensor_scalar_add(t1[:], n[:], EPS)          # n + eps
    t2 = pool.tile([P, F], f32, tag=tag + "_t2")
    nc.vector.tensor_scalar_add(t2[:], n2[:], 1.0)         # 1 + n2
    den = pool.tile([P, F], f32, tag=tag + "_den")
    nc.vector.tensor_mul(den[:], t1[:], t2[:])
    rec = pool.tile([P, F], f32, tag=tag + "_rec")
    nc.vector.reciprocal(rec[:], den[:])
    sc = pool.tile([P, F], f32, tag=tag + "_sc")
    nc.vector.tensor_mul(sc[:], n2[:], rec[:])
    return sc


def build(stage="full"):
    nc = bacc.Bacc("TRN2", target_bir_lowering=False, debug=False,
                   num_devices=N_CORES)

    xp = nc.declare_dram_parameter("xp", [BPC, K1, S1], f32, isOutput=False)
    w1s = nc.declare_dram_parameter("w1s", [K1, 128], f32, isOutput=False)
    pcw = nc.declare_dram_parameter("pcw", [OFF, 128, 128], f32, isOutput=False)
    pcb = nc.declare_dram_parameter("pcb", [128, 1], f32, isOutput=False)
    rw = nc.declare_dram_parameter("rw", [C, 128, SLO * CAP * O], f32, isOutput=False)
    w1d = nc.declare_dram_parameter("w1d", [33, D1], f32, isOutput=False)
    w2d = nc.declare_dram_parameter("w2d", [D1 + 1, D2], f32, isOutput=False)
    w3d = nc.declare_dram_parameter("w3d", [D2, D3PC], f32, isOutput=False)
    b3d = nc.declare_dram_parameter("b3d", [1, D3PC], f32, isOutput=False)
    w4d = nc.declare_dram_parameter("w4d", [KT4, 128, D4PC], f32, isOutput=False)
    b4d = nc.declare_dram_parameter("b4d", [1, D4PC], f32, isOutput=False)
    idt = nc.declare_dram_parameter("idt", [16, 16], f32, isOutput=False)

    cls_out = nc.declare_dram_parameter("cls_out", [BPC, C], f32, isOutput=True)
    rec_out = nc.declare_dram_parameter("rec_out", [B, D4PC], f32, isOutput=True)

    dbg = None
    if stage != "full":
        dbg_shapes = {
            "h": [BPC, 128, S1],
            "p": [BPC, 128, SP],
            "u": [BPC, 128, CAP * SLO],
            "pr": [BPC, 128, C * O * SLO],
            "v": [BPC, 128, C * O],
            "d": [B, 32],
            "h1": [128, 4 * 16],
            "h3": [KT4, 128, 16],
        }
        dbg = nc.declare_dram_parameter("dbg", dbg_shapes[stage], f32, isOutput=True)

    with tile.TileContext(nc) as tc:
        _emit(nc, tc, locals(), stage)
    nc.compile()
    return nc


def _emit(nc, tc, T, stage):
    xp, w1s, pcw, pcb, rw = T["xp"], T["w1s"], T["pcw"], T["pcb"], T["rw"]
    w1d, w2d, w3d, b3d, w4d, b4d, idt = (T["w1d"], T["w2d"], T["w3d"],
                                         T["b3d"], T["w4d"], T["b4d"], T["idt"])
    cls_out, rec_out, dbg = T["cls_out"], T["rec_out"], T["dbg"]

    import contextlib
    ctx = contextlib.ExitStack()
    with ctx:
        misc = ctx.enter_context(tc.tile_pool(name="misc", bufs=1))
        dram = ctx.enter_context(tc.tile_pool(name="dram", bufs=1, space="DRAM"))

        # --- small persistent tiles ---
        w1s_t = misc.tile([K1, 128], f32)
        nc.sync.dma_start(w1s_t[:], w1s[:])
        pcb_t = misc.tile([128, 1], f32)
        nc.sync.dma_start(pcb_t[:], pcb[:])
        idt_t = misc.tile([16, 16], f32)
        nc.sync.dma_start(idt_t[:], idt[:])
        ones128 = misc.tile([128, 1], f32)
        nc.vector.memset(ones128[:], 1.0)
        halves128 = misc.tile([128, 1], f32)
        nc.vector.memset(halves128[:], 0.5)
        ones1 = misc.tile([1, 128], f32)
        nc.vector.memset(ones1[:], 1.0)

        u_ts = [misc.tile([128, CAP * SLO], f32, tag=f"u{b}", name=f"u{b}") for b in range(BPC)]

        # ================= Phase A: conv1 + pcaps + squash (per batch) ======
        with (
            tc.tile_pool(name="conv", bufs=2) as convp,
            tc.tile_pool(name="hpool", bufs=1) as hp,
            tc.tile_pool(name="wring", bufs=3) as wring,
            tc.tile_pool(name="cps", bufs=4, space="PSUM") as cps,
            tc.tile_pool(name="pps", bufs=1, space="PSUM") as pps,
            tc.tile_pool(name="sq", bufs=1) as sq,
        ):
            for b in range(BPC):
                h_t = hp.tile([128, S1], f32, tag="h")
                # ---- conv1: K=28 matmuls over im2col patches ----
                XC = 6561  # spatial chunk (3 chunks of 6561)
                for ci in range(3):
                    xp_t = convp.tile([K1, XC], f32, tag="xp")
                    nc.sync.dma_start(xp_t[:], xp[b, :, ci * XC:(ci + 1) * XC])
                    n0 = 0
                    while n0 < XC:
                        nn = min(512, XC - n0)
                        cp = cps.tile([128, 512], f32, tag="c1")
                        nc.tensor.matmul(cp[:, :nn], w1s_t[:], xp_t[:, n0:n0 + nn],
                                         start=True, stop=True)
                        # relu straight into h (alternate ACT/DVE for balance)
                        dst = h_t[:, ci * XC + n0: ci * XC + n0 + nn]
                        if (n0 // 512) % 2 == 0:
                            nc.scalar.activation(dst, cp[:, :nn], AF.Relu)
                        else:
                            nc.vector.tensor_relu(dst, cp[:, :nn])
                        n0 += nn

                if stage == "h":
                    nc.sync.dma_start(dbg[b], h_t[:])
                    continue

                # ---- pcaps: 729 accumulated K=128 matmuls ----
                pp0 = pps.tile([128, 500], f32, tag="pp0")
                pp1 = pps.tile([128, 500], f32, tag="pp1")
                pp_halves = [pp0, pp1]
                hr = h_t[:].rearrange("p (d h w) -> p d h w", d=27, h=27, w=27)
                for oc0 in range(0, OFF, WT_CHUNK):
                    ocn = min(WT_CHUNK, OFF - oc0)
                    wt = wring.tile([128, WT_CHUNK * 128], f32, tag="wt")
                    # pcw[oc0:oc0+ocn] : [ocn, 128, 128] -> SBUF [cin, (t, cout)]
                    wsrc = pcw[oc0:oc0 + ocn].rearrange("t k m -> k t m")
                    nc.sync.dma_start(wt[:, :ocn * 128], wsrc)
                    for t in range(ocn):
                        o = oc0 + t
                        kd, kh, kw = o // 81, (o // 9) % 9, o % 9
                        lhsT = wt[:, t * 128:(t + 1) * 128]
                        for half in range(2):
                            d0 = half * 5
                            rhs = hr[:, kd + 2 * d0:kd + 2 * d0 + 9:2,
                                     kh:kh + 19:2, kw:kw + 19:2]
                            nc.tensor.matmul(pp_halves[half][:],
                                             lhsT, rhs,
                                             start=(o == 0), stop=(o == OFF - 1),
                                             skip_group_check=True)
                # drain PSUM with per-cout bias add
                p_t = sq.tile([128, SP], f32, tag="p_t")
                nc.scalar.activation(p_t[:, 0:500], pp0[:], AF.Identity,
                                     bias=pcb_t[:, 0:1], scale=1.0)
                nc.scalar.activation(p_t[:, 500:1000], pp1[:], AF.Identity,
                                     bias=pcb_t[:, 0:1], scale=1.0)
                if stage == "p":
                    nc.sync.dma_start(dbg[b], p_t[:])
                    continue

                # ---- transpose p -> u_pre via DRAM bounce ----
                pd = dram.tile([128, SP], f32, tag="pd")
                nc.sync.dma_start(pd[:], p_t[:])
                up_t = sq.tile([128, CAP * SLO], f32, tag="up")
                pdr = pd[:].rearrange("(cap dim) (shi slo) -> dim shi cap slo",
                                      cap=CAP, dim=DIM, shi=SHI, slo=SLO)
                for d in range(DIM):
                    # out [8 part(shi), (cap, slo)]; in dims (shi, cap, slo)
                    dst = up_t[d * SHI:(d + 1) * SHI, :].rearrange(
                        "shi (cap slo) -> shi cap slo", cap=CAP, slo=SLO)
                    nc.sync.dma_start(dst, pdr[d].rearrange("shi cap slo -> shi cap slo"))

                # ---- squash over the 8 capsule types ----
                sqv = sq.tile([128, CAP * SLO], f32, tag="sqv")
                nc.scalar.activation(sqv[:], up_t[:], AF.Square)
                n2 = sq.tile([128, SLO], f32, tag="n2")
                nc.vector.tensor_reduce(
                    n2[:], sqv[:].rearrange("p (cap slo) -> p slo cap", cap=CAP),
                    mybir.AxisListType.X, ALU.add)
                sc = _squash_scale(nc, sq, n2, "sqs")
                u_t = u_ts[b]
                for cap in range(CAP):
                    nc.vector.tensor_mul(u_t[:, cap * SLO:(cap + 1) * SLO],
                                         up_t[:, cap * SLO:(cap + 1) * SLO], sc[:])

            if stage in ("h", "p"):
                return
            if stage == "u":
                for b in range(BPC):
                    nc.sync.dma_start(dbg[b], u_ts[b][:])
                return

        # ================= Phase B: priors + routing ========================
        dd = dram.tile([BPC, 32], f32, tag="dd")
        cls_sb = [misc.tile([128, C], f32, tag=f"cls{b}", name=f"cls{b}") for b in range(BPC)]
        with tc.tile_pool(name="rpr", bufs=1) as rp:
            pr_ts = []
            with tc.tile_pool(name="rwp", bufs=1) as rwp:
                rw_t = rwp.tile([128, C * SLO * CAP * O], f32, tag="rw_t")
                for c in range(C):
                    nc.sync.dma_start(rw_t[:, c * 16000:(c + 1) * 16000], rw[c])
                rwv = rw_t[:].rearrange("p (c slo cap o) -> p c o slo cap",
                                        c=C, slo=SLO, cap=CAP, o=O)
                for b in range(BPC):
                    u_t = u_ts[b]
                    uv = u_t[:].rearrange("p (cap slo) -> p slo cap", cap=CAP)
                    pr_t = rp.tile([128, C * O * SLO], f32, tag=f"pr{b}", name=f"pr{b}")
                    pr_ts.append(pr_t)
                    prod = rp.tile([128, SLO * CAP], f32, tag="prod")
                    prodv = prod[:].rearrange("p (slo cap) -> p slo cap", cap=CAP)
                    for c in range(C):
                        for o in range(O):
                            nc.vector.tensor_mul(prodv, uv, rwv[:, c, o])
                            nc.vector.tensor_reduce(
                                pr_t[:, (c * O + o) * SLO:(c * O + o + 1) * SLO],
                                prodv, mybir.AxisListType.X, ALU.add)

            if stage == "pr":
                for b in range(BPC):
                    nc.sync.dma_start(dbg[b], pr_ts[b][:])
                return

            # routing iterations
            with (
                tc.tile_pool(name="rit", bufs=1) as ri,
                tc.tile_pool(name="rps", bufs=2, space="PSUM") as rps,
            ):
                for b in range(BPC):
                    pr_t = pr_ts[b]
                    lg = ri.tile([128, C * SLO], f32, tag="lg")
                    S_part = ri.tile([128, C * O], f32, tag="S_part")
                    v_t = None
                    for it in range(3):
                        if it == 0:
                            # probs = 0.5 -> S = 0.5 * sum_r priors (fold 0.5
                            # into the partition-reduce lhsT)
                            nc.vector.tensor_reduce(
                                S_part[:],
                                pr_t[:].rearrange("p (co slo) -> p co slo", slo=SLO),
                                mybir.AxisListType.X, ALU.add)
                            red_lhs = halves128
                        else:
                            pb0 = ri.tile([128, SLO], f32, tag="pb0")
                            nc.vector.tensor_sub(pb0[:], lg[:, 0:SLO], lg[:, SLO:2 * SLO])
                            nc.scalar.activation(pb0[:], pb0[:], AF.Sigmoid)
                            pb1 = ri.tile([128, SLO], f32, tag="pb1")
                            nc.scalar.activation(pb1[:], pb0[:], AF.Copy,
                                                 bias=1.0, scale=-1.0)
                            scr = ri.tile([128, SLO], f32, tag="scr")
                            for c in range(C):
                                pb = pb0 if c == 0 else pb1
                                for o in range(O):
                                    co = c * O + o
                                    nc.vector.scalar_tensor_tensor(
                                        scr[:], pr_t[:, co * SLO:(co + 1) * SLO],
                                        1.0, pb[:], ALU.bypass, ALU.mult,
                                        accum_out=S_part[:, co:co + 1])
                            red_lhs = ones128
                        S_ps = rps.tile([1, C * O], f32, tag="S_ps")
                        nc.tensor.matmul(S_ps[:], red_lhs[:], S_part[:],
                                         start=True, stop=True)
                        S_sb = ri.tile([1, C * O], f32, tag="S_sb")
                        nc.vector.tensor_copy(S_sb[:], S_ps[:])
                        vr_ps = rps.tile([128, C * O], f32, tag="vr_ps")
                        nc.tensor.matmul(vr_ps[:], ones1[:], S_sb[:],
                                         start=True, stop=True)
                        vraw = ri.tile([128, C * O], f32, tag="vraw")
                        nc.vector.tensor_copy(vraw[:], vr_ps[:])
                        # squash over o per class
                        vsq = ri.tile([128, C * O], f32, tag="vsq")
                        nc.scalar.activation(vsq[:], vraw[:], AF.Square)
                        n2v = ri.tile([128, C], f32, tag="n2v")
                        nc.vector.tensor_reduce(
                            n2v[:], vsq[:].rearrange("p (c o) -> p c o", c=C),
                            mybir.AxisListType.X, ALU.add)
                        scv = _squash_scale(nc, ri, n2v, "vs")
                        v_t = ri.tile([128, C * O], f32, tag="v_t")
                        for c in range(C):
                            nc.vector.tensor_scalar_mul(
                                v_t[:, c * O:(c + 1) * O],
                                vraw[:, c * O:(c + 1) * O], scv[:, c:c + 1])
                        if it < 2:
                            # logits += sum_o priors * v
                            for c in range(C):
                                for o in range(O):
                                    co = c * O + o
                                    pslice = pr_t[:, co * SLO:(co + 1) * SLO]
                                    lslice = lg[:, c * SLO:(c + 1) * SLO]
                                    if it == 0 and o == 0:
                                        nc.vector.tensor_scalar_mul(
                                            lslice, pslice, v_t[:, co:co + 1])
                                    else:
                                        nc.vector.scalar_tensor_tensor(
                                            lslice, pslice, v_t[:, co:co + 1],
                                            lslice, ALU.mult, ALU.add)

                    # classes = ||v|| ; d = one-hot(argmax) masked caps
                    vsqf = ri.tile([128, C * O], f32, tag="vsqf")
                    nc.scalar.activation(vsqf[:], v_t[:], AF.Square)
                    n2c = ri.tile([128, C], f32, tag="n2c")
                    nc.vector.tensor_reduce(
                        n2c[:], vsqf[:].rearrange("p (c o) -> p c o", c=C),
                        mybir.AxisListType.X, ALU.add)
                    nc.scalar.activation(cls_sb[b][:], n2c[:], AF.Sqrt)
                    nc.sync.dma_start(cls_out[b:b + 1, :], cls_sb[b][0:1, :])

                    m0 = ri.tile([1, 1], f32, tag="m0")
                    nc.vector.tensor_tensor(m0[:], cls_sb[b][0:1, 0:1],
                                            cls_sb[b][0:1, 1:2], ALU.is_ge)
                    m1 = ri.tile([1, 1], f32, tag="m1")
                    nc.scalar.activation(m1[:], m0[:], AF.Copy, bias=1.0, scale=-1.0)
                    d_row = ri.tile([1, 32], f32, tag="d_row")
                    nc.vector.tensor_scalar_mul(d_row[:, 0:16], v_t[0:1, 0:16], m0[:])
                    nc.vector.tensor_scalar_mul(d_row[:, 16:32], v_t[0:1, 16:32], m1[:])
                    nc.sync.dma_start(dd[b:b + 1, :], d_row[:])

        # ================= Phase C: decoder =================================
        Dsh = dram.tile([B, 32], f32, tag="Dsh")
        nc.gpsimd.collective_compute(
            "AllGather", ALU.bypass,
            replica_groups=[list(range(N_CORES))],
            ins=[dd[:]], outs=[Dsh[:]])
        if stage == "d":
            nc.sync.dma_start(dbg[:], Dsh[:])
            return

        with (
            tc.tile_pool(name="dec", bufs=1) as dp,
            tc.tile_pool(name="dring", bufs=2) as dr,
            tc.tile_pool(name="w4ring", bufs=2) as w4r,
        ):
            dps_ctx = tc.tile_pool(name="dps", bufs=1, space="PSUM")
            dps = dps_ctx.__enter__()
            D_sb = dp.tile([16, 32], f32, tag="D_sb")
            nc.sync.dma_start(D_sb[:], Dsh[:])
            DT_ps = dps.tile([32, 16], f32, tag="DT_ps")
            nc.tensor.transpose(DT_ps[:], D_sb[:], idt_t[:])
            DT33 = dp.tile([33, 16], f32, tag="DT33")
            nc.vector.tensor_copy(DT33[0:32, :], DT_ps[:])
            nc.vector.memset(DT33[32:33, :], 1.0)
            ones1_16 = dp.tile([1, 16], f32, tag="o116")
            nc.vector.memset(ones1_16[:], 1.0)

            # fc1: [33,16] -> [512,16] (transposed), K=33 incl. bias row
            w1d_t = dp.tile([33, D1], f32, tag="w1d_t")
            nc.sync.dma_start(w1d_t[:], w1d[:])
            r1 = dps.tile([128, 4 * 16], f32, tag="r1")
            h1t = dp.tile([128, 4 * 16], f32, tag="h1t")
            for jc in range(4):
                nc.tensor.matmul(r1[:, jc * 16:(jc + 1) * 16],
                                 w1d_t[:, jc * 128:(jc + 1) * 128], DT33[:],
                                 start=True, stop=True)
                nc.scalar.activation(h1t[:, jc * 16:(jc + 1) * 16],
                                     r1[:, jc * 16:(jc + 1) * 16], AF.Relu)
            if stage == "h1":
                nc.sync.dma_start(dbg[:], h1t[:])
                return

            # fc2: K=512 (+bias), 16 j-chunks, resident k-tiles
            w2k = dp.tile([128, 4 * D2], f32, tag="w2k")
            for kt in range(4):
                nc.sync.dma_start(w2k[:, kt * D2:(kt + 1) * D2],
                                  w2d[kt * 128:(kt + 1) * 128, :])
            w2b = dp.tile([1, D2], f32, tag="w2b")
            nc.sync.dma_start(w2b[:], w2d[D1:D1 + 1, :])
            r2 = dps.tile([128, 16 * 16], f32, tag="r2")
            h2t = dp.tile([128, 16 * 16], f32, tag="h2t")
            for jc in range(16):
                for kt in range(4):
                    nc.tensor.matmul(r2[:, jc * 16:(jc + 1) * 16],
                                     w2k[:, kt * D2 + jc * 128:kt * D2 + (jc + 1) * 128],
                                     h1t[:, kt * 16:(kt + 1) * 16],
                                     start=(kt == 0), stop=False,
                                     skip_group_check=True)
                nc.tensor.matmul(r2[:, jc * 16:(jc + 1) * 16],
                                 w2b[:, jc * 128:(jc + 1) * 128], ones1_16[:],
                                 start=False, stop=True, skip_group_check=True)
                nc.scalar.activation(h2t[:, jc * 16:(jc + 1) * 16],
                                     r2[:, jc * 16:(jc + 1) * 16], AF.Relu)

            dps_ctx.__exit__(None, None, None)
            # fc3: K=2048 (+bias), 10 j-chunks of this core's 1280 columns
            r3ps_ctx = tc.tile_pool(name="r3ps", bufs=1, space="PSUM")
            r3ps = r3ps_ctx.__enter__()
            w3k = dp.tile([128, 16 * D3PC], f32, tag="w3k")
            for kt in range(16):
                nc.sync.dma_start(w3k[:, kt * D3PC:(kt + 1) * D3PC],
                                  w3d[kt * 128:(kt + 1) * 128, :])
            b3t = dp.tile([1, D3PC], f32, tag="b3t")
            nc.sync.dma_start(b3t[:], b3d[:])
            r3 = r3ps.tile([128, 10 * 16], f32, tag="r3")
            h3t = dp.tile([128, 10 * 16], f32, tag="h3t")
            for jc in range(10):
                for kt in range(16):
                    nc.tensor.matmul(r3[:, jc * 16:(jc + 1) * 16],
                                     w3k[:, kt * D3PC + jc * 128:kt * D3PC + (jc + 1) * 128],
                                     h2t[:, kt * 16:(kt + 1) * 16],
                                     start=(kt == 0), stop=False,
                                     skip_group_check=True)
                nc.tensor.matmul(r3[:, jc * 16:(jc + 1) * 16],
                                 b3t[:, jc * 128:(jc + 1) * 128], ones1_16[:],
                                 start=False, stop=True, skip_group_check=True)
                nc.scalar.activation(h3t[:, jc * 16:(jc + 1) * 16],
                                     r3[:, jc * 16:(jc + 1) * 16], AF.Relu)

            # allgather fc3 activations: [10,128,16] -> [80,128,16]
            h3loc = dram.tile([10, 128, 16], f32, tag="h3loc")
            nc.sync.dma_start(
                h3loc[:].rearrange("jc p b -> p jc b"),
                h3t[:].rearrange("p (jc b) -> p jc b", jc=10))
            h3all = dram.tile([KT4, 128, 16], f32, tag="h3all")
            nc.gpsimd.collective_compute(
                "AllGather", ALU.bypass,
                replica_groups=[list(range(N_CORES))],
                ins=[h3loc[:]], outs=[h3all[:]])
            if stage == "h3":
                nc.sync.dma_start(dbg[:], h3all[:])
                return
            h3a = dp.tile([128, KT4 * 16], f32, tag="h3a")
            nc.sync.dma_start(
                h3a[:].rearrange("p (kt b) -> p kt b", kt=KT4),
                h3all[:].rearrange("kt p b -> p kt b"))

            r3ps_ctx.__exit__(None, None, None)
            # fc4: K=10240 (+bias), this core's 3375 columns, stream k-tiles
            r4ps_ctx = tc.tile_pool(name="r4ps", bufs=1, space="PSUM")
            r4ps = r4ps_ctx.__enter__()
            NCH = [512] * 6 + [303]
            r4 = [r4ps.tile([16, NCH[i]], f32, tag=f"r4_{i}", name=f"r4_{i}") for i in range(7)]
            for kt in range(KT4):
                w4t = w4r.tile([128, D4PC], f32, tag="w4t")
                nc.sync.dma_start(w4t[:], w4d[kt])
                n0 = 0
                for i, nn in enumerate(NCH):
                    nc.tensor.matmul(r4[i][:], h3a[:, kt * 16:(kt + 1) * 16],
                                     w4t[:, n0:n0 + nn],
                                     start=(kt == 0), stop=False,
                                     skip_group_check=True)
                    n0 += nn
            b4t = dp.tile([1, D4PC], f32, tag="b4t")
            nc.sync.dma_start(b4t[:], b4d[:])
            rec_sb = dp.tile([16, D4PC], f32, tag="rec_sb")
            n0 = 0
            for i, nn in enumerate(NCH):
                nc.tensor.matmul(r4[i][:], ones1_16[:], b4t[:, n0:n0 + nn],
                                 start=False, stop=True, skip_group_check=True)
                nc.scalar.activation(rec_sb[:, n0:n0 + nn], r4[i][:], AF.Sigmoid)
                n0 += nn
            nc.sync.dma_start(rec_out[:], rec_sb[:])
            r4ps_ctx.__exit__(None, None, None)


# ======================= host side =========================================

def _host_prep(inputs):
    x = np.asarray(inputs["x"], np.float32)
    conv1_w = np.asarray(inputs["conv1_w"], np.float32)
    conv1_b = np.asarray(inputs["conv1_b"], np.float32)
    pcaps_w = np.asarray(inputs["pcaps_w"], np.float32)
    pcaps_b = np.asarray(inputs["pcaps_b"], np.float32)
    route_w = np.asarray(inputs["route_w"], np.float32)

    # conv1 im2col patches + ones row (bias)
    xs = x[:, 0]  # [16,29,29,29]
    s0, s1, s2 = xs.strides[1:]
    patches = np.lib.stride_tricks.as_strided(
        xs, (B, 3, 3, 3, 27, 27, 27),
        (xs.strides[0], s0, s1, s2, s0, s1, s2))
    xp_full = np.empty((B, K1, S1), np.float32)
    xp_full[:, :27] = patches.reshape(B, 27, S1)
    xp_full[:, 27] = 1.0

    w1s = np.empty((K1, 128), np.float32)
    w1s[:27] = conv1_w.reshape(128, 27).T
    w1s[27] = conv1_b

    pcw = np.ascontiguousarray(
        pcaps_w.reshape(128, 128, OFF).transpose(2, 1, 0))
    pcb = pcaps_b.reshape(128, 1).copy()
    rwt = np.ascontiguousarray(route_w.reshape(C, 128, SLO * CAP * O))

    w1d = np.vstack([np.asarray(inputs["dec_w1"], np.float32),
                     np.asarray(inputs["dec_b1"], np.float32)[None]])
    w2d = np.vstack([np.asarray(inputs["dec_w2"], np.float32),
                     np.asarray(inputs["dec_b2"], np.float32)[None]])
    w3 = np.asarray(inputs["dec_w3"], np.float32)
    b3 = np.asarray(inputs["dec_b3"], np.float32)
    w4 = np.asarray(inputs["dec_w4"], np.float32)
    b4 = np.asarray(inputs["dec_b4"], np.float32)
    idt = np.eye(16, dtype=np.float32)

    in_maps = []
    for k in range(N_CORES):
        in_maps.append({
            "xp": np.ascontiguousarray(xp_full[2 * k:2 * k + 2]),
            "w1s": w1s, "pcw": pcw, "pcb": pcb, "rw": rwt,
            "w1d": w1d, "w2d": w2d,
            "w3d": np.ascontiguousarray(w3[:, k * D3PC:(k + 1) * D3PC]),
            "b3d": np.ascontiguousarray(b3[None, k * D3PC:(k + 1) * D3PC]),
            "w4d": np.ascontiguousarray(
                w4[:, k * D4PC:(k + 1) * D4PC]).reshape(KT4, 128, D4PC),
            "b4d": np.ascontiguousarray(b4[None, k * D4PC:(k + 1) * D4PC]),
            "idt": idt,
        })
    return in_maps


def run(inputs, stage="full", trace=False):
    key = stage
    if key not in _CACHE:
        _CACHE[key] = build(stage)
    nc = _CACHE[key]
    in_maps = _host_prep(inputs)
    res = run_bass_kernel_spmd(nc, in_maps, core_ids=list(range(N_CORES)),
                               trace=trace)
    return res


def kernel(**inputs):
    res = run(inputs, stage="full")
    classes = np.concatenate([res.results[k]["cls_out"] for k in range(N_CORES)], 0)
    recon = np.concatenate([res.results[k]["rec_out"] for k in range(N_CORES)], 1)
    return classes, recon


# revision 8
# speedup vs baseline: 1.9773x; 1.9773x over previous
"""Trainium2 Bass kernel for nn_CapsuleNet_3D (8 NeuronCores, SPMD).

Strategy:
  - Data-parallel over batch (16) for conv stem / primary caps / routing:
    2 batch elements per core.
  - conv1 (1->128, k=3, valid) as one K=28 matmul group over host-built
    im2col patches (27 patch rows + 1 ones row folding the bias).
  - primary caps conv (128->128, k=9, s=2) as 729 accumulated K=128
    matmuls in PSUM, with host-transposed weights [o, cin, cout] streamed
    through SBUF.
  - squash / routing priors / 3 dynamic-routing iterations on DVE/ACT with
    r=16000 laid out as [128 partitions x 125].
  - decoder fc1/fc2 replicated, fc3/fc4 column-split across the 8 cores
    (1280 / 3375 columns each); d-vector and fc3 activations exchanged with
    AllGather collectives; fc4 weights (138MB/core) streamed from HBM.
Outputs: classes [16,2], recon [16,27000].
"""

import numpy as np

import concourse.bass as bass
import concourse.bacc as bacc
import concourse.mybir as mybir
from concourse import tile
from concourse.bass_utils import run_bass_kernel_spmd

f32 = mybir.dt.float32
f16 = mybir.dt.float16
import os as _os
PCDT = {"f32": mybir.dt.float32, "f16": mybir.dt.float16,
        "bf16": mybir.dt.bfloat16}[_os.environ.get("PCDT", "f16")]
PC_NP = {"f32": np.float32, "f16": np.float16,
         "bf16": None}[_os.environ.get("PCDT", "f16")]
AF = mybir.ActivationFunctionType
ALU = mybir.AluOpType

N_CORES = 8
B = 16
BPC = 2              # batch per core
S1 = 27 * 27 * 27    # conv1 output spatial (19683)
K1 = 28              # 27 kernel taps + bias row
OFF = 729            # pcaps kernel taps (9^3)
SP = 1000            # pcaps output spatial (10^3)
DIM, SHI, SLO, CAP, O, C = 16, 8, 125, 8, 16, 2
D1, D2, D3, D4 = 512, 2048, 10240, 27000
D3PC, D4PC = D3 // N_CORES, D4 // N_CORES   # 1280, 3375
KT4 = D3 // 128      # 80 k-tiles for fc4
WT_CHUNK = 27        # pcaps weight taps per streamed chunk (27 | 729)
EPS = 1e-8

_CACHE = {}


def _squash_scale(nc, pool, n2, tag):
    """Given n2 = |t|^2 [P, F] (SBUF), return scale = n2/(1+n2)/(sqrt(n2)+eps)."""
    P, F = n2.shape
    n = pool.tile([P, F], f32, tag=tag + "_n")
    nc.scalar.activation(n[:], n2[:], AF.Sqrt)
    t1 = pool.tile([P, F], f32, tag=tag + "_t1")
    nc.vector.tensor_scalar_add(t1[:], n[:], EPS)          # n + eps
    t2 = pool.tile([P, F], f32, tag=tag + "_t2")
    nc.vector.tensor_scalar_add(t2[:], n2[:], 1.0)         # 1 + n2
    den = pool.tile([P, F], f32, tag=tag + "_den")
    nc.vector.tensor_mul(den[:], t1[:], t2[:])
    rec = pool.tile([P, F], f32, tag=tag + "_rec")
    nc.vector.reciprocal(rec[:], den[:])
    sc = pool.tile([P, F], f32, tag=tag + "_sc")
    nc.vector.tensor_mul(sc[:], n2[:], rec[:])
    return sc


def build(stage="full"):
    nc = bacc.Bacc("TRN2", target_bir_lowering=False, debug=False,
                   num_devices=N_CORES)

    xp = nc.declare_dram_parameter("xp", [BPC, K1, S1], f32, isOutput=False)
    w1s = nc.declare_dram_parameter("w1s", [K1, 128], f32, isOutput=False)
    pcw = nc.declare_dram_parameter("pcw", [OFF, 128, 128], PCDT, isOutput=False)
    pcb = nc.declare_dram_parameter("pcb", [128, 1], f32, isOutput=False)
    rw = nc.declare_dram_parameter("rw", [C, 128, SLO * CAP * O], f32, isOutput=False)
    w1d = nc.declare_dram_parameter("w1d", [33, D1], f32, isOutput=False)
    w2d = nc.declare_dram_parameter("w2d", [D1 + 1, D2], f32, isOutput=False)
    w3d = nc.declare_dram_parameter("w3d", [D2, D3PC], f32, isOutput=False)
    b3d = nc.declare_dram_parameter("b3d", [1, D3PC], f32, isOutput=False)
    w4d = nc.declare_dram_parameter("w4d", [KT4, 128, D4PC], f32, isOutput=False)
    b4d = nc.declare_dram_parameter("b4d", [1, D4PC], f32, isOutput=False)
    idt = nc.declare_dram_parameter("idt", [16, 16], f32, isOutput=False)

    cls_out = nc.declare_dram_parameter("cls_out", [BPC, C], f32, isOutput=True)
    rec_out = nc.declare_dram_parameter("rec_out", [B, D4PC], f32, isOutput=True)

    dbg = None
    if stage != "full":
        dbg_shapes = {
            "h": [BPC, 128, S1],
            "p": [BPC, 128, SP],
            "u": [BPC, 128, CAP * SLO],
            "pr": [BPC, 128, C * O * SLO],
            "v": [BPC, 128, C * O],
            "d": [B, 32],
            "h1": [128, 4 * 16],
            "h3": [KT4, 128, 16],
        }
        dbg = nc.declare_dram_parameter("dbg", dbg_shapes[stage], f32, isOutput=True)

    with tile.TileContext(nc) as tc:
        _emit(nc, tc, locals(), stage)
    nc.compile()
    return nc


def _emit(nc, tc, T, stage):
    xp, w1s, pcw, pcb, rw = T["xp"], T["w1s"], T["pcw"], T["pcb"], T["rw"]
    w1d, w2d, w3d, b3d, w4d, b4d, idt = (T["w1d"], T["w2d"], T["w3d"],
                                         T["b3d"], T["w4d"], T["b4d"], T["idt"])
    cls_out, rec_out, dbg = T["cls_out"], T["rec_out"], T["dbg"]

    import contextlib
    ctx = contextlib.ExitStack()
    with ctx:
        misc = ctx.enter_context(tc.tile_pool(name="misc", bufs=1))
        dram = ctx.enter_context(tc.tile_pool(name="dram", bufs=1, space="DRAM"))

        # --- small persistent tiles ---
        w1s_t = misc.tile([K1, 128], f32)
        nc.sync.dma_start(w1s_t[:], w1s[:])
        pcb_t = misc.tile([128, 1], f32)
        nc.sync.dma_start(pcb_t[:], pcb[:])
        idt_t = misc.tile([16, 16], f32)
        nc.sync.dma_start(idt_t[:], idt[:])
        ones128 = misc.tile([128, 1], f32)
        nc.vector.memset(ones128[:], 1.0)
        halves128 = misc.tile([128, 1], f32)
        nc.vector.memset(halves128[:], 0.5)
        ones1 = misc.tile([1, 128], f32)
        nc.vector.memset(ones1[:], 1.0)

        u_ts = [misc.tile([128, CAP * SLO], f32, tag=f"u{b}", name=f"u{b}") for b in range(BPC)]

        # ================= Phase A: conv1 + pcaps + squash (per batch) ======
        with (
            tc.tile_pool(name="conv", bufs=2) as convp,
            tc.tile_pool(name="hpool", bufs=1) as hp,
            tc.tile_pool(name="wring", bufs=3) as wring,
            tc.tile_pool(name="cps", bufs=4, space="PSUM") as cps,
            tc.tile_pool(name="pps", bufs=1, space="PSUM") as pps,
            tc.tile_pool(name="sq", bufs=1) as sq,
        ):
            for b in range(BPC):
                _sid, _ = nc.enter_named_scope(f"conv1_{b}", False)
                h_t = hp.tile([128, S1], PCDT, tag="h")
                # ---- conv1: K=28 matmuls over im2col patches ----
                XC = 6561  # spatial chunk (3 chunks of 6561)
                for ci in range(3):
                    xp_t = convp.tile([K1, XC], f32, tag="xp")
                    nc.sync.dma_start(xp_t[:], xp[b, :, ci * XC:(ci + 1) * XC])
                    n0 = 0
                    while n0 < XC:
                        nn = min(512, XC - n0)
                        cp = cps.tile([128, 512], f32, tag="c1")
                        nc.tensor.matmul(cp[:, :nn], w1s_t[:], xp_t[:, n0:n0 + nn],
                                         start=True, stop=True)
                        # relu straight into h (alternate ACT/DVE for balance)
                        dst = h_t[:, ci * XC + n0: ci * XC + n0 + nn]
                        if (n0 // 512) % 2 == 0:
                            nc.scalar.activation(dst, cp[:, :nn], AF.Relu)
                        else:
                            nc.vector.tensor_relu(dst, cp[:, :nn])
                        n0 += nn

                nc.leave_named_scope(f"conv1_{b}", _sid, False)
                if stage == "h":
                    nc.sync.dma_start(dbg[b], h_t[:])
                    continue

                # ---- pcaps: 729 accumulated K=128 matmuls ----
                _sid, _ = nc.enter_named_scope(f"pcaps_{b}", False)
                pp0 = pps.tile([128, 500], f32, tag="pp0")
                pp1 = pps.tile([128, 500], f32, tag="pp1")
                pp_halves = [pp0, pp1]
                hr = h_t[:].rearrange("p (d h w) -> p d h w", d=27, h=27, w=27)
                for oc0 in range(0, OFF, WT_CHUNK):
                    ocn = min(WT_CHUNK, OFF - oc0)
                    wt = wring.tile([128, WT_CHUNK * 128], PCDT, tag="wt")
                    # pcw[oc0:oc0+ocn] : [ocn, 128, 128] -> SBUF [cin, (t, cout)]
                    wsrc = pcw[oc0:oc0 + ocn].rearrange("t k m -> k t m")
                    nc.sync.dma_start(wt[:, :ocn * 128], wsrc)
                    for t in range(ocn):
                        o = oc0 + t
                        kd, kh, kw = o // 81, (o // 9) % 9, o % 9
                        lhsT = wt[:, t * 128:(t + 1) * 128]
                        for half in range(2):
                            d0 = half * 5
                            rhs = hr[:, kd + 2 * d0:kd + 2 * d0 + 9:2,
                                     kh:kh + 19:2, kw:kw + 19:2]
                            nc.tensor.matmul(pp_halves[half][:],
                                             lhsT, rhs,
                                             start=(o == 0), stop=(o == OFF - 1),
                                             skip_group_check=True)
                # drain PSUM with per-cout bias add
                p_t = sq.tile([128, SP], f32, tag="p_t")
                nc.scalar.activation(p_t[:, 0:500], pp0[:], AF.Identity,
                                     bias=pcb_t[:, 0:1], scale=1.0)
                nc.scalar.activation(p_t[:, 500:1000], pp1[:], AF.Identity,
                                     bias=pcb_t[:, 0:1], scale=1.0)
                nc.leave_named_scope(f"pcaps_{b}", _sid, False)
                if stage == "p":
                    nc.sync.dma_start(dbg[b], p_t[:])
                    continue

                # ---- transpose p -> u_pre via DRAM bounce ----
                _sid, _ = nc.enter_named_scope(f"squash_{b}", False)
                pd = dram.tile([128, SP], f32, tag="pd")
                nc.sync.dma_start(pd[:], p_t[:])
                up_t = sq.tile([128, CAP * SLO], f32, tag="up")
                pdr = pd[:].rearrange("(cap dim) (shi slo) -> dim shi cap slo",
                                      cap=CAP, dim=DIM, shi=SHI, slo=SLO)
                for d in range(DIM):
                    # out [8 part(shi), (cap, slo)]; in dims (shi, cap, slo)
                    dst = up_t[d * SHI:(d + 1) * SHI, :].rearrange(
                        "shi (cap slo) -> shi cap slo", cap=CAP, slo=SLO)
                    nc.sync.dma_start(dst, pdr[d].rearrange("shi cap slo -> shi cap slo"))

                # ---- squash over the 8 capsule types ----
                sqv = sq.tile([128, CAP * SLO], f32, tag="sqv")
                nc.scalar.activation(sqv[:], up_t[:], AF.Square)
                n2 = sq.tile([128, SLO], f32, tag="n2")
                nc.vector.tensor_reduce(
                    n2[:], sqv[:].rearrange("p (cap slo) -> p slo cap", cap=CAP),
                    mybir.AxisListType.X, ALU.add)
                sc = _squash_scale(nc, sq, n2, "sqs")
                u_t = u_ts[b]
                for cap in range(CAP):
                    nc.vector.tensor_mul(u_t[:, cap * SLO:(cap + 1) * SLO],
                                         up_t[:, cap * SLO:(cap + 1) * SLO], sc[:])
                nc.leave_named_scope(f"squash_{b}", _sid, False)

            if stage in ("h", "p"):
                return
            if stage == "u":
                for b in range(BPC):
                    nc.sync.dma_start(dbg[b], u_ts[b][:])
                return

        # ================= Phase B: priors + routing ========================
        dd = dram.tile([BPC, 32], f32, tag="dd")
        cls_sb = [misc.tile([128, C], f32, tag=f"cls{b}", name=f"cls{b}") for b in range(BPC)]
        _prsid, _ = nc.enter_named_scope("priors", False)
        with tc.tile_pool(name="rpr", bufs=1) as rp:
            pr_ts = []
            with tc.tile_pool(name="rwp", bufs=1) as rwp:
                rw_t = rwp.tile([128, C * SLO * CAP * O], f32, tag="rw_t")
                for c in range(C):
                    nc.sync.dma_start(rw_t[:, c * 16000:(c + 1) * 16000], rw[c])
                rwv = rw_t[:].rearrange("p (c slo cap o) -> p c o slo cap",
                                        c=C, slo=SLO, cap=CAP, o=O)
                for b in range(BPC):
                    u_t = u_ts[b]
                    uv = u_t[:].rearrange("p (cap slo) -> p slo cap", cap=CAP)
                    pr_t = rp.tile([128, C * O * SLO], f32, tag=f"pr{b}", name=f"pr{b}")
                    pr_ts.append(pr_t)
                    prod = rp.tile([128, SLO * CAP], f32, tag="prod")
                    prodv = prod[:].rearrange("p (slo cap) -> p slo cap", cap=CAP)
                    for c in range(C):
                        for o in range(O):
                            nc.vector.tensor_mul(prodv, uv, rwv[:, c, o])
                            nc.vector.tensor_reduce(
                                pr_t[:, (c * O + o) * SLO:(c * O + o + 1) * SLO],
                                prodv, mybir.AxisListType.X, ALU.add)

            nc.leave_named_scope("priors", _prsid, False)
            if stage == "pr":
                for b in range(BPC):
                    nc.sync.dma_start(dbg[b], pr_ts[b][:])
                return

            # routing iterations
            _rtsid, _ = nc.enter_named_scope("routing", False)
            with (
                tc.tile_pool(name="rit", bufs=1) as ri,
                tc.tile_pool(name="rps", bufs=2, space="PSUM") as rps,
            ):
                for b in range(BPC):
                    pr_t = pr_ts[b]
                    lg = ri.tile([128, C * SLO], f32, tag="lg")
                    S_part = ri.tile([128, C * O], f32, tag="S_part")
                    v_t = None
                    for it in range(3):
                        if it == 0:
                            # probs = 0.5 -> S = 0.5 * sum_r priors (fold 0.5
                            # into the partition-reduce lhsT)
                            nc.vector.tensor_reduce(
                                S_part[:],
                                pr_t[:].rearrange("p (co slo) -> p co slo", slo=SLO),
                                mybir.AxisListType.X, ALU.add)
                            red_lhs = halves128
                        else:
                            pb0 = ri.tile([128, SLO], f32, tag="pb0")
                            nc.vector.tensor_sub(pb0[:], lg[:, 0:SLO], lg[:, SLO:2 * SLO])
                            nc.scalar.activation(pb0[:], pb0[:], AF.Sigmoid)
                            pb1 = ri.tile([128, SLO], f32, tag="pb1")
                            nc.scalar.activation(pb1[:], pb0[:], AF.Copy,
                                                 bias=1.0, scale=-1.0)
                            scr = ri.tile([128, SLO], f32, tag="scr")
                            for c in range(C):
                                pb = pb0 if c == 0 else pb1
                                for o in range(O):
                                    co = c * O + o
                                    nc.vector.scalar_tensor_tensor(
                                        scr[:], pr_t[:, co * SLO:(co + 1) * SLO],
                                        1.0, pb[:], ALU.bypass, ALU.mult,
                                        accum_out=S_part[:, co:co + 1])
                            red_lhs = ones128
                        S_ps = rps.tile([1, C * O], f32, tag="S_ps")
                        nc.tensor.matmul(S_ps[:], red_lhs[:], S_part[:],
                                         start=True, stop=True)
                        S_sb = ri.tile([1, C * O], f32, tag="S_sb")
                        nc.vector.tensor_copy(S_sb[:], S_ps[:])
                        vr_ps = rps.tile([128, C * O], f32, tag="vr_ps")
                        nc.tensor.matmul(vr_ps[:], ones1[:], S_sb[:],
                                         start=True, stop=True)
                        vraw = ri.tile([128, C * O], f32, tag="vraw")
                        nc.vector.tensor_copy(vraw[:], vr_ps[:])
                        # squash over o per class
                        vsq = ri.tile([128, C * O], f32, tag="vsq")
                        nc.scalar.activation(vsq[:], vraw[:], AF.Square)
                        n2v = ri.tile([128, C], f32, tag="n2v")
                        nc.vector.tensor_reduce(
                            n2v[:], vsq[:].rearrange("p (c o) -> p c o", c=C),
                            mybir.AxisListType.X, ALU.add)
                        scv = _squash_scale(nc, ri, n2v, "vs")
                        v_t = ri.tile([128, C * O], f32, tag="v_t")
                        for c in range(C):
                            nc.vector.tensor_scalar_mul(
                                v_t[:, c * O:(c + 1) * O],
                                vraw[:, c * O:(c + 1) * O], scv[:, c:c + 1])
                        if it < 2:
                            # logits += sum_o priors * v
                            for c in range(C):
                                for o in range(O):
                                    co = c * O + o
                                    pslice = pr_t[:, co * SLO:(co + 1) * SLO]
                                    lslice = lg[:, c * SLO:(c + 1) * SLO]
                                    if it == 0 and o == 0:
                                        nc.vector.tensor_scalar_mul(
                                            lslice, pslice, v_t[:, co:co + 1])
                                    else:
                                        nc.vector.scalar_tensor_tensor(
                                            lslice, pslice, v_t[:, co:co + 1],
                                            lslice, ALU.mult, ALU.add)

                    # classes = ||v|| ; d = one-hot(argmax) masked caps
                    vsqf = ri.tile([128, C * O], f32, tag="vsqf")
                    nc.scalar.activation(vsqf[:], v_t[:], AF.Square)
                    n2c = ri.tile([128, C], f32, tag="n2c")
                    nc.vector.tensor_reduce(
                        n2c[:], vsqf[:].rearrange("p (c o) -> p c o", c=C),
                        mybir.AxisListType.X, ALU.add)
                    nc.scalar.activation(cls_sb[b][:], n2c[:], AF.Sqrt)
                    nc.sync.dma_start(cls_out[b:b + 1, :], cls_sb[b][0:1, :])

                    m0 = ri.tile([1, 1], f32, tag="m0")
                    nc.vector.tensor_tensor(m0[:], cls_sb[b][0:1, 0:1],
                                            cls_sb[b][0:1, 1:2], ALU.is_ge)
                    m1 = ri.tile([1, 1], f32, tag="m1")
                    nc.scalar.activation(m1[:], m0[:], AF.Copy, bias=1.0, scale=-1.0)
                    d_row = ri.tile([1, 32], f32, tag="d_row")
                    nc.vector.tensor_scalar_mul(d_row[:, 0:16], v_t[0:1, 0:16], m0[:])
                    nc.vector.tensor_scalar_mul(d_row[:, 16:32], v_t[0:1, 16:32], m1[:])
                    nc.sync.dma_start(dd[b:b + 1, :], d_row[:])
            nc.leave_named_scope("routing", _rtsid, False)

        # ================= Phase C: decoder =================================
        _agsid, _ = nc.enter_named_scope("ag_d", False)
        Dsh = dram.tile([B, 32], f32, tag="Dsh")
        nc.gpsimd.collective_compute(
            "AllGather", ALU.bypass,
            replica_groups=[list(range(N_CORES))],
            ins=[dd[:]], outs=[Dsh[:]])
        nc.leave_named_scope("ag_d", _agsid, False)
        if stage == "d":
            nc.sync.dma_start(dbg[:], Dsh[:])
            return

        with (
            tc.tile_pool(name="dec", bufs=1) as dp,
            tc.tile_pool(name="dring", bufs=2) as dr,
            tc.tile_pool(name="w4ring", bufs=2) as w4r,
        ):
            _fcsid, _ = nc.enter_named_scope("fc123", False)
            dps_ctx = tc.tile_pool(name="dps", bufs=1, space="PSUM")
            dps = dps_ctx.__enter__()
            D_sb = dp.tile([16, 32], f32, tag="D_sb")
            nc.sync.dma_start(D_sb[:], Dsh[:])
            DT_ps = dps.tile([32, 16], f32, tag="DT_ps")
            nc.tensor.transpose(DT_ps[:], D_sb[:], idt_t[:])
            DT33 = dp.tile([33, 16], f32, tag="DT33")
            nc.vector.tensor_copy(DT33[0:32, :], DT_ps[:])
            nc.vector.memset(DT33[32:33, :], 1.0)
            ones1_16 = dp.tile([1, 16], f32, tag="o116")
            nc.vector.memset(ones1_16[:], 1.0)

            # fc1: [33,16] -> [512,16] (transposed), K=33 incl. bias row
            w1d_t = dp.tile([33, D1], f32, tag="w1d_t")
            nc.sync.dma_start(w1d_t[:], w1d[:])
            r1 = dps.tile([128, 4 * 16], f32, tag="r1")
            h1t = dp.tile([128, 4 * 16], f32, tag="h1t")
            for jc in range(4):
                nc.tensor.matmul(r1[:, jc * 16:(jc + 1) * 16],
                                 w1d_t[:, jc * 128:(jc + 1) * 128], DT33[:],
                                 start=True, stop=True)
                nc.scalar.activation(h1t[:, jc * 16:(jc + 1) * 16],
                                     r1[:, jc * 16:(jc + 1) * 16], AF.Relu)
            if stage == "h1":
                nc.sync.dma_start(dbg[:], h1t[:])
                return

            # fc2: K=512 (+bias), 16 j-chunks, resident k-tiles
            w2k = dp.tile([128, 4 * D2], f32, tag="w2k")
            for kt in range(4):
                nc.sync.dma_start(w2k[:, kt * D2:(kt + 1) * D2],
                                  w2d[kt * 128:(kt + 1) * 128, :])
            w2b = dp.tile([1, D2], f32, tag="w2b")
            nc.sync.dma_start(w2b[:], w2d[D1:D1 + 1, :])
            r2 = dps.tile([128, 16 * 16], f32, tag="r2")
            h2t = dp.tile([128, 16 * 16], f32, tag="h2t")
            for jc in range(16):
                for kt in range(4):
                    nc.tensor.matmul(r2[:, jc * 16:(jc + 1) * 16],
                                     w2k[:, kt * D2 + jc * 128:kt * D2 + (jc + 1) * 128],
                                     h1t[:, kt * 16:(kt + 1) * 16],
                                     start=(kt == 0), stop=False,
                                     skip_group_check=True)
                nc.tensor.matmul(r2[:, jc * 16:(jc + 1) * 16],
                                 w2b[:, jc * 128:(jc + 1) * 128], ones1_16[:],
                                 start=False, stop=True, skip_group_check=True)
                nc.scalar.activation(h2t[:, jc * 16:(jc + 1) * 16],
                                     r2[:, jc * 16:(jc + 1) * 16], AF.Relu)

            dps_ctx.__exit__(None, None, None)
            # fc3: K=2048 (+bias), 10 j-chunks of this core's 1280 columns
            r3ps_ctx = tc.tile_pool(name="r3ps", bufs=1, space="PSUM")
            r3ps = r3ps_ctx.__enter__()
            w3k = dp.tile([128, 16 * D3PC], f32, tag="w3k")
            for kt in range(16):
                nc.sync.dma_start(w3k[:, kt * D3PC:(kt + 1) * D3PC],
                                  w3d[kt * 128:(kt + 1) * 128, :])
            b3t = dp.tile([1, D3PC], f32, tag="b3t")
            nc.sync.dma_start(b3t[:], b3d[:])
            r3 = r3ps.tile([128, 10 * 16], f32, tag="r3")
            h3t = dp.tile([128, 10 * 16], f32, tag="h3t")
            for jc in range(10):
                for kt in range(16):
                    nc.tensor.matmul(r3[:, jc * 16:(jc + 1) * 16],
                                     w3k[:, kt * D3PC + jc * 128:kt * D3PC + (jc + 1) * 128],
                                     h2t[:, kt * 16:(kt + 1) * 16],
                                     start=(kt == 0), stop=False,
                                     skip_group_check=True)
                nc.tensor.matmul(r3[:, jc * 16:(jc + 1) * 16],
                                 b3t[:, jc * 128:(jc + 1) * 128], ones1_16[:],
                                 start=False, stop=True, skip_group_check=True)
                nc.scalar.activation(h3t[:, jc * 16:(jc + 1) * 16],
                                     r3[:, jc * 16:(jc + 1) * 16], AF.Relu)

            nc.leave_named_scope("fc123", _fcsid, False)
            _agsid2, _ = nc.enter_named_scope("ag_h3", False)
            # allgather fc3 activations: [10,128,16] -> [80,128,16]
            h3loc = dram.tile([10, 128, 16], f32, tag="h3loc")
            nc.sync.dma_start(
                h3loc[:].rearrange("jc p b -> p jc b"),
                h3t[:].rearrange("p (jc b) -> p jc b", jc=10))
            h3all = dram.tile([KT4, 128, 16], f32, tag="h3all")
            nc.gpsimd.collective_compute(
                "AllGather", ALU.bypass,
                replica_groups=[list(range(N_CORES))],
                ins=[h3loc[:]], outs=[h3all[:]])
            if stage == "h3":
                nc.sync.dma_start(dbg[:], h3all[:])
                return
            nc.leave_named_scope("ag_h3", _agsid2, False)
            _fc4sid, _ = nc.enter_named_scope("fc4", False)
            h3a = dp.tile([128, KT4 * 16], f32, tag="h3a")
            nc.sync.dma_start(
                h3a[:].rearrange("p (kt b) -> p kt b", kt=KT4),
                h3all[:].rearrange("kt p b -> p kt b"))

            r3ps_ctx.__exit__(None, None, None)
            # fc4: K=10240 (+bias), this core's 3375 columns, stream k-tiles
            r4ps_ctx = tc.tile_pool(name="r4ps", bufs=1, space="PSUM")
            r4ps = r4ps_ctx.__enter__()
            NCH = [512] * 6 + [303]
            r4 = [r4ps.tile([16, NCH[i]], f32, tag=f"r4_{i}", name=f"r4_{i}") for i in range(7)]
            for kt in range(KT4):
                w4t = w4r.tile([128, D4PC], f32, tag="w4t")
                nc.sync.dma_start(w4t[:], w4d[kt])
                n0 = 0
                for i, nn in enumerate(NCH):
                    nc.tensor.matmul(r4[i][:], h3a[:, kt * 16:(kt + 1) * 16],
                                     w4t[:, n0:n0 + nn],
                                     start=(kt == 0), stop=False,
                                     skip_group_check=True)
                    n0 += nn
            b4t = dp.tile([1, D4PC], f32, tag="b4t")
            nc.sync.dma_start(b4t[:], b4d[:])
            rec_sb = dp.tile([16, D4PC], f32, tag="rec_sb")
            n0 = 0
            for i, nn in enumerate(NCH):
                nc.tensor.matmul(r4[i][:], ones1_16[:], b4t[:, n0:n0 + nn],
                                 start=False, stop=True, skip_group_check=True)
                nc.scalar.activation(rec_sb[:, n0:n0 + nn], r4[i][:], AF.Sigmoid)
                n0 += nn
            nc.sync.dma_start(rec_out[:], rec_sb[:])
            nc.leave_named_scope("fc4", _fc4sid, False)
            r4ps_ctx.__exit__(None, None, None)


# ======================= host side =========================================

def _host_prep(inputs):
    x = np.asarray(inputs["x"], np.float32)
    conv1_w = np.asarray(inputs["conv1_w"], np.float32)
    conv1_b = np.asarray(inputs["conv1_b"], np.float32)
    pcaps_w = np.asarray(inputs["pcaps_w"], np.float32)
    pcaps_b = np.asarray(inputs["pcaps_b"], np.float32)
    route_w = np.asarray(inputs["route_w"], np.float32)

    # conv1 im2col patches + ones row (bias)
    xs = x[:, 0]  # [16,29,29,29]
    s0, s1, s2 = xs.strides[1:]
    patches = np.lib.stride_tricks.as_strided(
        xs, (B, 3, 3, 3, 27, 27, 27),
        (xs.strides[0], s0, s1, s2, s0, s1, s2))
    xp_full = np.empty((B, K1, S1), np.float32)
    xp_full[:, :27] = patches.reshape(B, 27, S1)
    xp_full[:, 27] = 1.0

    w1s = np.empty((K1, 128), np.float32)
    w1s[:27] = conv1_w.reshape(128, 27).T
    w1s[27] = conv1_b

    pcw = np.ascontiguousarray(
        pcaps_w.reshape(128, 128, OFF).transpose(2, 1, 0))
    if PC_NP is not None and PC_NP is not np.float32:
        pcw = pcw.astype(PC_NP)
    elif PC_NP is None:
        import ml_dtypes
        pcw = pcw.astype(ml_dtypes.bfloat16)
    pcb = pcaps_b.reshape(128, 1).copy()
    rwt = np.ascontiguousarray(route_w.reshape(C, 128, SLO * CAP * O))

    w1d = np.vstack([np.asarray(inputs["dec_w1"], np.float32),
                     np.asarray(inputs["dec_b1"], np.float32)[None]])
    w2d = np.vstack([np.asarray(inputs["dec_w2"], np.float32),
                     np.asarray(inputs["dec_b2"], np.float32)[None]])
    w3 = np.asarray(inputs["dec_w3"], np.float32)
    b3 = np.asarray(inputs["dec_b3"], np.float32)
    w4 = np.asarray(inputs["dec_w4"], np.float32)
    b4 = np.asarray(inputs["dec_b4"], np.float32)
    idt = np.eye(16, dtype=np.float32)

    in_maps = []
    for k in range(N_CORES):
        in_maps.append({
            "xp": np.ascontiguousarray(xp_full[2 * k:2 * k + 2]),
            "w1s": w1s, "pcw": pcw, "pcb": pcb, "rw": rwt,
            "w1d": w1d, "w2d": w2d,
            "w3d": np.ascontiguousarray(w3[:, k * D3PC:(k + 1) * D3PC]),
            "b3d": np.ascontiguousarray(b3[None, k * D3PC:(k + 1) * D3PC]),
            "w4d": np.ascontiguousarray(
                w4[:, k * D4PC:(k + 1) * D4PC]).reshape(KT4, 128, D4PC),
            "b4d": np.ascontiguousarray(b4[None, k * D4PC:(k + 1) * D4PC]),
            "idt": idt,
        })
    return in_maps


def run(inputs, stage="full", trace=False):
    key = stage
    if key not in _CACHE:
        _CACHE[key] = build(stage)
    nc = _CACHE[key]
    in_maps = _host_prep(inputs)
    res = run_bass_kernel_spmd(nc, in_maps, core_ids=list(range(N_CORES)),
                               trace=trace)
    return res


def kernel(**inputs):
    res = run(inputs, stage="full")
    classes = np.concatenate([res.results[k]["cls_out"] for k in range(N_CORES)], 0)
    recon = np.concatenate([res.results[k]["rec_out"] for k in range(N_CORES)], 1)
    return classes, recon


# revision 9
# speedup vs baseline: 2.5456x; 1.2874x over previous
"""Trainium2 Bass kernel for nn_CapsuleNet_3D (8 NeuronCores, SPMD).

Strategy:
  - Data-parallel over batch (16) for conv stem / primary caps / routing:
    2 batch elements per core.
  - conv1 (1->128, k=3, valid) as one K=28 matmul group over host-built
    im2col patches (27 patch rows + 1 ones row folding the bias).
  - primary caps conv (128->128, k=9, s=2) as 729 accumulated K=128
    matmuls in PSUM, with host-transposed weights [o, cin, cout] streamed
    through SBUF.
  - squash / routing priors / 3 dynamic-routing iterations on DVE/ACT with
    r=16000 laid out as [128 partitions x 125].
  - decoder fc1/fc2 replicated, fc3/fc4 column-split across the 8 cores
    (1280 / 3375 columns each); d-vector and fc3 activations exchanged with
    AllGather collectives; fc4 weights (138MB/core) streamed from HBM.
Outputs: classes [16,2], recon [16,27000].
"""

import numpy as np

import concourse.bass as bass
import concourse.bacc as bacc
import concourse.mybir as mybir
from concourse import tile
from concourse.bass_utils import run_bass_kernel_spmd

f32 = mybir.dt.float32
f16 = mybir.dt.float16
import os as _os
PCDT = {"f32": mybir.dt.float32, "f16": mybir.dt.float16,
        "bf16": mybir.dt.bfloat16}[_os.environ.get("PCDT", "f16")]
PC_NP = {"f32": np.float32, "f16": np.float16,
         "bf16": None}[_os.environ.get("PCDT", "f16")]
AF = mybir.ActivationFunctionType
ALU = mybir.AluOpType

N_CORES = 8
B = 16
BPC = 2              # batch per core
S1 = 27 * 27 * 27    # conv1 output spatial (19683)
K1 = 28              # 27 kernel taps + bias row
OFF = 729            # pcaps kernel taps (9^3)
SP = 1000            # pcaps output spatial (10^3)
DIM, SHI, SLO, CAP, O, C = 16, 8, 125, 8, 16, 2
D1, D2, D3, D4 = 512, 2048, 10240, 27000
D3PC, D4PC = D3 // N_CORES, D4 // N_CORES   # 1280, 3375
KT4 = D3 // 128      # 80 k-tiles for fc4
WT_CHUNK = 27        # pcaps weight taps per streamed chunk (27 | 729)
EPS = 1e-8

_CACHE = {}


def _squash_scale(nc, pool, n2, tag):
    """Given n2 = |t|^2 [P, F] (SBUF), return scale = n2/(1+n2)/(sqrt(n2)+eps)."""
    P, F = n2.shape
    n = pool.tile([P, F], f32, tag=tag + "_n")
    nc.scalar.activation(n[:], n2[:], AF.Sqrt)
    t1 = pool.tile([P, F], f32, tag=tag + "_t1")
    nc.vector.tensor_scalar_add(t1[:], n[:], EPS)          # n + eps
    t2 = pool.tile([P, F], f32, tag=tag + "_t2")
    nc.vector.tensor_scalar_add(t2[:], n2[:], 1.0)         # 1 + n2
    den = pool.tile([P, F], f32, tag=tag + "_den")
    nc.vector.tensor_mul(den[:], t1[:], t2[:])
    rec = pool.tile([P, F], f32, tag=tag + "_rec")
    nc.vector.reciprocal(rec[:], den[:])
    sc = pool.tile([P, F], f32, tag=tag + "_sc")
    nc.vector.tensor_mul(sc[:], n2[:], rec[:])
    return sc


def build(stage="full"):
    nc = bacc.Bacc("TRN2", target_bir_lowering=False, debug=False,
                   num_devices=N_CORES)

    xp = nc.declare_dram_parameter("xp", [BPC, K1, S1], f32, isOutput=False)
    w1s = nc.declare_dram_parameter("w1s", [K1, 128], f32, isOutput=False)
    pcw = nc.declare_dram_parameter("pcw", [OFF, 128, 128], PCDT, isOutput=False)
    pcb = nc.declare_dram_parameter("pcb", [128, 1], f32, isOutput=False)
    rw = nc.declare_dram_parameter("rw", [C, 128, SLO * CAP * O], f32, isOutput=False)
    w1d = nc.declare_dram_parameter("w1d", [33, D1], f16, isOutput=False)
    w2d = nc.declare_dram_parameter("w2d", [D1 + 1, D2], f16, isOutput=False)
    w3d = nc.declare_dram_parameter("w3d", [D2, D3PC], f16, isOutput=False)
    b3d = nc.declare_dram_parameter("b3d", [1, D3PC], f16, isOutput=False)
    w4d = nc.declare_dram_parameter("w4d", [KT4, 128, D4PC], f16, isOutput=False)
    b4d = nc.declare_dram_parameter("b4d", [1, D4PC], f16, isOutput=False)
    idt = nc.declare_dram_parameter("idt", [16, 16], f32, isOutput=False)

    cls_out = nc.declare_dram_parameter("cls_out", [BPC, C], f32, isOutput=True)
    rec_out = nc.declare_dram_parameter("rec_out", [B, D4PC], f32, isOutput=True)

    dbg = None
    if stage != "full":
        dbg_shapes = {
            "h": [BPC, 128, S1],
            "p": [BPC, 128, SP],
            "u": [BPC, 128, CAP * SLO],
            "pr": [BPC, 128, C * O * SLO],
            "v": [BPC, 128, C * O],
            "d": [B, 32],
            "h1": [128, 4 * 16],
            "h3": [KT4, 128, 16],
        }
        dbg = nc.declare_dram_parameter("dbg", dbg_shapes[stage], f32, isOutput=True)

    with tile.TileContext(nc) as tc:
        _emit(nc, tc, locals(), stage)
    nc.compile()
    return nc


def _emit(nc, tc, T, stage):
    xp, w1s, pcw, pcb, rw = T["xp"], T["w1s"], T["pcw"], T["pcb"], T["rw"]
    w1d, w2d, w3d, b3d, w4d, b4d, idt = (T["w1d"], T["w2d"], T["w3d"],
                                         T["b3d"], T["w4d"], T["b4d"], T["idt"])
    cls_out, rec_out, dbg = T["cls_out"], T["rec_out"], T["dbg"]

    import contextlib
    ctx = contextlib.ExitStack()
    with ctx:
        misc = ctx.enter_context(tc.tile_pool(name="misc", bufs=1))
        dram = ctx.enter_context(tc.tile_pool(name="dram", bufs=1, space="DRAM"))

        # --- small persistent tiles ---
        w1s_t = misc.tile([K1, 128], f32)
        nc.sync.dma_start(w1s_t[:], w1s[:])
        pcb_t = misc.tile([128, 1], f32)
        nc.sync.dma_start(pcb_t[:], pcb[:])
        idt_t = misc.tile([16, 16], f32)
        nc.sync.dma_start(idt_t[:], idt[:])
        ones128 = misc.tile([128, 1], f32)
        nc.vector.memset(ones128[:], 1.0)
        halves128 = misc.tile([128, 1], f32)
        nc.vector.memset(halves128[:], 0.5)
        ones1 = misc.tile([1, 128], f32)
        nc.vector.memset(ones1[:], 1.0)

        u_ts = [misc.tile([128, CAP * SLO], f32, tag=f"u{b}", name=f"u{b}") for b in range(BPC)]

        # ================= Phase A: conv1 + pcaps + squash (per batch) ======
        with (
            tc.tile_pool(name="conv", bufs=2) as convp,
            tc.tile_pool(name="hpool", bufs=1) as hp,
            tc.tile_pool(name="wring", bufs=3) as wring,
            tc.tile_pool(name="cps", bufs=4, space="PSUM") as cps,
            tc.tile_pool(name="pps", bufs=1, space="PSUM") as pps,
            tc.tile_pool(name="sq", bufs=1) as sq,
        ):
            for b in range(BPC):
                _sid, _ = nc.enter_named_scope(f"conv1_{b}", False)
                h_t = hp.tile([128, S1], PCDT, tag="h")
                # ---- conv1: K=28 matmuls over im2col patches ----
                XC = 6561  # spatial chunk (3 chunks of 6561)
                for ci in range(3):
                    xp_t = convp.tile([K1, XC], f32, tag="xp")
                    nc.sync.dma_start(xp_t[:], xp[b, :, ci * XC:(ci + 1) * XC])
                    n0 = 0
                    while n0 < XC:
                        nn = min(512, XC - n0)
                        cp = cps.tile([128, 512], f32, tag="c1")
                        nc.tensor.matmul(cp[:, :nn], w1s_t[:], xp_t[:, n0:n0 + nn],
                                         start=True, stop=True)
                        # relu straight into h (alternate ACT/DVE for balance)
                        dst = h_t[:, ci * XC + n0: ci * XC + n0 + nn]
                        if (n0 // 512) % 2 == 0:
                            nc.scalar.activation(dst, cp[:, :nn], AF.Relu)
                        else:
                            nc.vector.tensor_relu(dst, cp[:, :nn])
                        n0 += nn

                nc.leave_named_scope(f"conv1_{b}", _sid, False)
                if stage == "h":
                    nc.sync.dma_start(dbg[b], h_t[:])
                    continue

                # ---- pcaps: 729 accumulated K=128 matmuls ----
                _sid, _ = nc.enter_named_scope(f"pcaps_{b}", False)
                pp0 = pps.tile([128, 500], f32, tag="pp0")
                pp1 = pps.tile([128, 500], f32, tag="pp1")
                pp_halves = [pp0, pp1]
                hr = h_t[:].rearrange("p (d h w) -> p d h w", d=27, h=27, w=27)
                for oc0 in range(0, OFF, WT_CHUNK):
                    ocn = min(WT_CHUNK, OFF - oc0)
                    wt = wring.tile([128, WT_CHUNK * 128], PCDT, tag="wt")
                    # pcw[oc0:oc0+ocn] : [ocn, 128, 128] -> SBUF [cin, (t, cout)]
                    wsrc = pcw[oc0:oc0 + ocn].rearrange("t k m -> k t m")
                    nc.sync.dma_start(wt[:, :ocn * 128], wsrc)
                    for t in range(ocn):
                        o = oc0 + t
                        kd, kh, kw = o // 81, (o // 9) % 9, o % 9
                        lhsT = wt[:, t * 128:(t + 1) * 128]
                        for half in range(2):
                            d0 = half * 5
                            rhs = hr[:, kd + 2 * d0:kd + 2 * d0 + 9:2,
                                     kh:kh + 19:2, kw:kw + 19:2]
                            nc.tensor.matmul(pp_halves[half][:],
                                             lhsT, rhs,
                                             start=(o == 0), stop=(o == OFF - 1),
                                             skip_group_check=True)
                # drain PSUM with per-cout bias add
                p_t = sq.tile([128, SP], f32, tag="p_t")
                nc.scalar.activation(p_t[:, 0:500], pp0[:], AF.Identity,
                                     bias=pcb_t[:, 0:1], scale=1.0)
                nc.scalar.activation(p_t[:, 500:1000], pp1[:], AF.Identity,
                                     bias=pcb_t[:, 0:1], scale=1.0)
                nc.leave_named_scope(f"pcaps_{b}", _sid, False)
                if stage == "p":
                    nc.sync.dma_start(dbg[b], p_t[:])
                    continue

                # ---- transpose p -> u_pre via DRAM bounce ----
                _sid, _ = nc.enter_named_scope(f"squash_{b}", False)
                pd = dram.tile([128, SP], f32, tag="pd")
                nc.sync.dma_start(pd[:], p_t[:])
                up_t = sq.tile([128, CAP * SLO], f32, tag="up")
                pdr = pd[:].rearrange("(cap dim) (shi slo) -> dim shi cap slo",
                                      cap=CAP, dim=DIM, shi=SHI, slo=SLO)
                for d in range(DIM):
                    # out [8 part(shi), (cap, slo)]; in dims (shi, cap, slo)
                    dst = up_t[d * SHI:(d + 1) * SHI, :].rearrange(
                        "shi (cap slo) -> shi cap slo", cap=CAP, slo=SLO)
                    nc.sync.dma_start(dst, pdr[d].rearrange("shi cap slo -> shi cap slo"))

                # ---- squash over the 8 capsule types ----
                sqv = sq.tile([128, CAP * SLO], f32, tag="sqv")
                nc.scalar.activation(sqv[:], up_t[:], AF.Square)
                n2 = sq.tile([128, SLO], f32, tag="n2")
                nc.vector.tensor_reduce(
                    n2[:], sqv[:].rearrange("p (cap slo) -> p slo cap", cap=CAP),
                    mybir.AxisListType.X, ALU.add)
                sc = _squash_scale(nc, sq, n2, "sqs")
                u_t = u_ts[b]
                for cap in range(CAP):
                    nc.vector.tensor_mul(u_t[:, cap * SLO:(cap + 1) * SLO],
                                         up_t[:, cap * SLO:(cap + 1) * SLO], sc[:])
                nc.leave_named_scope(f"squash_{b}", _sid, False)

            if stage in ("h", "p"):
                return
            if stage == "u":
                for b in range(BPC):
                    nc.sync.dma_start(dbg[b], u_ts[b][:])
                return

        # ================= Phase B: priors + routing ========================
        dd = dram.tile([BPC, 32], f32, tag="dd")
        cls_sb = [misc.tile([128, C], f32, tag=f"cls{b}", name=f"cls{b}") for b in range(BPC)]
        _prsid, _ = nc.enter_named_scope("priors", False)
        with tc.tile_pool(name="rpr", bufs=1) as rp:
            pr_ts = []
            with tc.tile_pool(name="rwp", bufs=1) as rwp:
                rw_t = rwp.tile([128, C * SLO * CAP * O], f32, tag="rw_t")
                for c in range(C):
                    nc.sync.dma_start(rw_t[:, c * 16000:(c + 1) * 16000], rw[c])
                rwv = rw_t[:].rearrange("p (c slo cap o) -> p c o slo cap",
                                        c=C, slo=SLO, cap=CAP, o=O)
                for b in range(BPC):
                    u_t = u_ts[b]
                    uv = u_t[:].rearrange("p (cap slo) -> p slo cap", cap=CAP)
                    pr_t = rp.tile([128, C * O * SLO], f32, tag=f"pr{b}", name=f"pr{b}")
                    pr_ts.append(pr_t)
                    prod = rp.tile([128, SLO * CAP], f32, tag="prod")
                    prodv = prod[:].rearrange("p (slo cap) -> p slo cap", cap=CAP)
                    for c in range(C):
                        for o in range(O):
                            nc.vector.tensor_mul(prodv, uv, rwv[:, c, o])
                            nc.vector.tensor_reduce(
                                pr_t[:, (c * O + o) * SLO:(c * O + o + 1) * SLO],
                                prodv, mybir.AxisListType.X, ALU.add)

            nc.leave_named_scope("priors", _prsid, False)
            if stage == "pr":
                for b in range(BPC):
                    nc.sync.dma_start(dbg[b], pr_ts[b][:])
                return

            # routing iterations
            _rtsid, _ = nc.enter_named_scope("routing", False)
            with (
                tc.tile_pool(name="rit", bufs=1) as ri,
                tc.tile_pool(name="rps", bufs=2, space="PSUM") as rps,
            ):
                for b in range(BPC):
                    pr_t = pr_ts[b]
                    lg = ri.tile([128, C * SLO], f32, tag="lg")
                    S_part = ri.tile([128, C * O], f32, tag="S_part")
                    v_t = None
                    for it in range(3):
                        if it == 0:
                            # probs = 0.5 -> S = 0.5 * sum_r priors (fold 0.5
                            # into the partition-reduce lhsT)
                            nc.vector.tensor_reduce(
                                S_part[:],
                                pr_t[:].rearrange("p (co slo) -> p co slo", slo=SLO),
                                mybir.AxisListType.X, ALU.add)
                            red_lhs = halves128
                        else:
                            pb0 = ri.tile([128, SLO], f32, tag="pb0")
                            nc.vector.tensor_sub(pb0[:], lg[:, 0:SLO], lg[:, SLO:2 * SLO])
                            nc.scalar.activation(pb0[:], pb0[:], AF.Sigmoid)
                            pb1 = ri.tile([128, SLO], f32, tag="pb1")
                            nc.scalar.activation(pb1[:], pb0[:], AF.Copy,
                                                 bias=1.0, scale=-1.0)
                            scr = ri.tile([128, SLO], f32, tag="scr")
                            for c in range(C):
                                pb = pb0 if c == 0 else pb1
                                for o in range(O):
                                    co = c * O + o
                                    nc.vector.scalar_tensor_tensor(
                                        scr[:], pr_t[:, co * SLO:(co + 1) * SLO],
                                        1.0, pb[:], ALU.bypass, ALU.mult,
                                        accum_out=S_part[:, co:co + 1])
                            red_lhs = ones128
                        S_ps = rps.tile([1, C * O], f32, tag="S_ps")
                        nc.tensor.matmul(S_ps[:], red_lhs[:], S_part[:],
                                         start=True, stop=True)
                        S_sb = ri.tile([1, C * O], f32, tag="S_sb")
                        nc.vector.tensor_copy(S_sb[:], S_ps[:])
                        vr_ps = rps.tile([128, C * O], f32, tag="vr_ps")
                        nc.tensor.matmul(vr_ps[:], ones1[:], S_sb[:],
                                         start=True, stop=True)
                        vraw = ri.tile([128, C * O], f32, tag="vraw")
                        nc.vector.tensor_copy(vraw[:], vr_ps[:])
                        # squash over o per class
                        vsq = ri.tile([128, C * O], f32, tag="vsq")
                        nc.scalar.activation(vsq[:], vraw[:], AF.Square)
                        n2v = ri.tile([128, C], f32, tag="n2v")
                        nc.vector.tensor_reduce(
                            n2v[:], vsq[:].rearrange("p (c o) -> p c o", c=C),
                            mybir.AxisListType.X, ALU.add)
                        scv = _squash_scale(nc, ri, n2v, "vs")
                        v_t = ri.tile([128, C * O], f32, tag="v_t")
                        for c in range(C):
                            nc.vector.tensor_scalar_mul(
                                v_t[:, c * O:(c + 1) * O],
                                vraw[:, c * O:(c + 1) * O], scv[:, c:c + 1])
                        if it < 2:
                            # logits += sum_o priors * v
                            for c in range(C):
                                for o in range(O):
                                    co = c * O + o
                                    pslice = pr_t[:, co * SLO:(co + 1) * SLO]
                                    lslice = lg[:, c * SLO:(c + 1) * SLO]
                                    if it == 0 and o == 0:
                                        nc.vector.tensor_scalar_mul(
                                            lslice, pslice, v_t[:, co:co + 1])
                                    else:
                                        nc.vector.scalar_tensor_tensor(
                                            lslice, pslice, v_t[:, co:co + 1],
                                            lslice, ALU.mult, ALU.add)

                    # classes = ||v|| ; d = one-hot(argmax) masked caps
                    vsqf = ri.tile([128, C * O], f32, tag="vsqf")
                    nc.scalar.activation(vsqf[:], v_t[:], AF.Square)
                    n2c = ri.tile([128, C], f32, tag="n2c")
                    nc.vector.tensor_reduce(
                        n2c[:], vsqf[:].rearrange("p (c o) -> p c o", c=C),
                        mybir.AxisListType.X, ALU.add)
                    nc.scalar.activation(cls_sb[b][:], n2c[:], AF.Sqrt)
                    nc.sync.dma_start(cls_out[b:b + 1, :], cls_sb[b][0:1, :])

                    m0 = ri.tile([1, 1], f32, tag="m0")
                    nc.vector.tensor_tensor(m0[:], cls_sb[b][0:1, 0:1],
                                            cls_sb[b][0:1, 1:2], ALU.is_ge)
                    m1 = ri.tile([1, 1], f32, tag="m1")
                    nc.scalar.activation(m1[:], m0[:], AF.Copy, bias=1.0, scale=-1.0)
                    d_row = ri.tile([1, 32], f32, tag="d_row")
                    nc.vector.tensor_scalar_mul(d_row[:, 0:16], v_t[0:1, 0:16], m0[:])
                    nc.vector.tensor_scalar_mul(d_row[:, 16:32], v_t[0:1, 16:32], m1[:])
                    nc.sync.dma_start(dd[b:b + 1, :], d_row[:])
            nc.leave_named_scope("routing", _rtsid, False)

        # ================= Phase C: decoder =================================
        _agsid, _ = nc.enter_named_scope("ag_d", False)
        Dsh = dram.tile([B, 32], f32, tag="Dsh")
        nc.gpsimd.collective_compute(
            "AllGather", ALU.bypass,
            replica_groups=[list(range(N_CORES))],
            ins=[dd[:]], outs=[Dsh[:]])
        nc.leave_named_scope("ag_d", _agsid, False)
        if stage == "d":
            nc.sync.dma_start(dbg[:], Dsh[:])
            return

        with (
            tc.tile_pool(name="dec", bufs=1) as dp,
            tc.tile_pool(name="dring", bufs=2) as dr,
            tc.tile_pool(name="w4ring", bufs=6) as w4r,
        ):
            _fcsid, _ = nc.enter_named_scope("fc123", False)
            dps_ctx = tc.tile_pool(name="dps", bufs=1, space="PSUM")
            dps = dps_ctx.__enter__()
            D_sb = dp.tile([16, 32], f32, tag="D_sb")
            nc.sync.dma_start(D_sb[:], Dsh[:])
            DT_ps = dps.tile([32, 16], f32, tag="DT_ps")
            nc.tensor.transpose(DT_ps[:], D_sb[:], idt_t[:])
            DT33 = dp.tile([33, 16], f16, tag="DT33")
            nc.vector.tensor_copy(DT33[0:32, :], DT_ps[:])
            nc.vector.memset(DT33[32:33, :], 1.0)
            ones1_16 = dp.tile([1, 16], f16, tag="o116")
            nc.vector.memset(ones1_16[:], 1.0)

            # fc1: [33,16] -> [512,16] (transposed), K=33 incl. bias row
            w1d_t = dp.tile([33, D1], f16, tag="w1d_t")
            nc.sync.dma_start(w1d_t[:], w1d[:])
            r1 = dps.tile([128, 4 * 16], f32, tag="r1")
            h1t = dp.tile([128, 4 * 16], f16, tag="h1t")
            for jc in range(4):
                nc.tensor.matmul(r1[:, jc * 16:(jc + 1) * 16],
                                 w1d_t[:, jc * 128:(jc + 1) * 128], DT33[:],
                                 start=True, stop=True)
                nc.scalar.activation(h1t[:, jc * 16:(jc + 1) * 16],
                                     r1[:, jc * 16:(jc + 1) * 16], AF.Relu)
            if stage == "h1":
                nc.sync.dma_start(dbg[:], h1t[:])
                return

            # fc2: K=512 (+bias), 16 j-chunks, resident k-tiles
            w2k = dp.tile([128, 4 * D2], f16, tag="w2k")
            for kt in range(4):
                nc.sync.dma_start(w2k[:, kt * D2:(kt + 1) * D2],
                                  w2d[kt * 128:(kt + 1) * 128, :])
            w2b = dp.tile([1, D2], f16, tag="w2b")
            nc.sync.dma_start(w2b[:], w2d[D1:D1 + 1, :])
            r2 = dps.tile([128, 16 * 16], f32, tag="r2")
            h2t = dp.tile([128, 16 * 16], f16, tag="h2t")
            for jc in range(16):
                for kt in range(4):
                    nc.tensor.matmul(r2[:, jc * 16:(jc + 1) * 16],
                                     w2k[:, kt * D2 + jc * 128:kt * D2 + (jc + 1) * 128],
                                     h1t[:, kt * 16:(kt + 1) * 16],
                                     start=(kt == 0), stop=False,
                                     skip_group_check=True)
                nc.tensor.matmul(r2[:, jc * 16:(jc + 1) * 16],
                                 w2b[:, jc * 128:(jc + 1) * 128], ones1_16[:],
                                 start=False, stop=True, skip_group_check=True)
                nc.scalar.activation(h2t[:, jc * 16:(jc + 1) * 16],
                                     r2[:, jc * 16:(jc + 1) * 16], AF.Relu)

            dps_ctx.__exit__(None, None, None)
            # fc3: K=2048 (+bias), 10 j-chunks of this core's 1280 columns
            r3ps_ctx = tc.tile_pool(name="r3ps", bufs=1, space="PSUM")
            r3ps = r3ps_ctx.__enter__()
            w3k = dp.tile([128, 16 * D3PC], f16, tag="w3k")
            for kt in range(16):
                nc.sync.dma_start(w3k[:, kt * D3PC:(kt + 1) * D3PC],
                                  w3d[kt * 128:(kt + 1) * 128, :])
            b3t = dp.tile([1, D3PC], f16, tag="b3t")
            nc.sync.dma_start(b3t[:], b3d[:])
            r3 = r3ps.tile([128, 10 * 16], f32, tag="r3")
            h3t = dp.tile([128, 10 * 16], f16, tag="h3t")
            for jc in range(10):
                for kt in range(16):
                    nc.tensor.matmul(r3[:, jc * 16:(jc + 1) * 16],
                                     w3k[:, kt * D3PC + jc * 128:kt * D3PC + (jc + 1) * 128],
                                     h2t[:, kt * 16:(kt + 1) * 16],
                                     start=(kt == 0), stop=False,
                                     skip_group_check=True)
                nc.tensor.matmul(r3[:, jc * 16:(jc + 1) * 16],
                                 b3t[:, jc * 128:(jc + 1) * 128], ones1_16[:],
                                 start=False, stop=True, skip_group_check=True)
                nc.scalar.activation(h3t[:, jc * 16:(jc + 1) * 16],
                                     r3[:, jc * 16:(jc + 1) * 16], AF.Relu)

            nc.leave_named_scope("fc123", _fcsid, False)
            _agsid2, _ = nc.enter_named_scope("ag_h3", False)
            # allgather fc3 activations: [10,128,16] -> [80,128,16]
            h3loc = dram.tile([10, 128, 16], f16, tag="h3loc")
            nc.sync.dma_start(
                h3loc[:].rearrange("jc p b -> p jc b"),
                h3t[:].rearrange("p (jc b) -> p jc b", jc=10))
            h3all = dram.tile([KT4, 128, 16], f16, tag="h3all")
            nc.gpsimd.collective_compute(
                "AllGather", ALU.bypass,
                replica_groups=[list(range(N_CORES))],
                ins=[h3loc[:]], outs=[h3all[:]])
            if stage == "h3":
                nc.sync.dma_start(dbg[:], h3all[:])
                return
            nc.leave_named_scope("ag_h3", _agsid2, False)
            _fc4sid, _ = nc.enter_named_scope("fc4", False)
            h3a = dp.tile([128, KT4 * 16], f16, tag="h3a")
            nc.sync.dma_start(
                h3a[:].rearrange("p (kt b) -> p kt b", kt=KT4),
                h3all[:].rearrange("kt p b -> p kt b"))

            r3ps_ctx.__exit__(None, None, None)
            # fc4: K=10240 (+bias), this core's 3375 columns, stream k-tiles
            r4ps_ctx = tc.tile_pool(name="r4ps", bufs=1, space="PSUM")
            r4ps = r4ps_ctx.__enter__()
            NCH = [512] * 6 + [303]
            r4 = [r4ps.tile([16, NCH[i]], f32, tag=f"r4_{i}", name=f"r4_{i}") for i in range(7)]
            for kt in range(KT4):
                w4t = w4r.tile([128, D4PC], f16, tag="w4t")
                nc.sync.dma_start(w4t[:], w4d[kt])
                n0 = 0
                for i, nn in enumerate(NCH):
                    nc.tensor.matmul(r4[i][:], h3a[:, kt * 16:(kt + 1) * 16],
                                     w4t[:, n0:n0 + nn],
                                     start=(kt == 0), stop=False,
                                     skip_group_check=True)
                    n0 += nn
            b4t = dp.tile([1, D4PC], f16, tag="b4t")
            nc.sync.dma_start(b4t[:], b4d[:])
            rec_sb = dp.tile([16, D4PC], f32, tag="rec_sb")
            n0 = 0
            for i, nn in enumerate(NCH):
                nc.tensor.matmul(r4[i][:], ones1_16[:], b4t[:, n0:n0 + nn],
                                 start=False, stop=True, skip_group_check=True)
                nc.scalar.activation(rec_sb[:, n0:n0 + nn], r4[i][:], AF.Sigmoid)
                n0 += nn
            nc.sync.dma_start(rec_out[:], rec_sb[:])
            nc.leave_named_scope("fc4", _fc4sid, False)
            r4ps_ctx.__exit__(None, None, None)


# ======================= host side =========================================

def _host_prep(inputs):
    x = np.asarray(inputs["x"], np.float32)
    conv1_w = np.asarray(inputs["conv1_w"], np.float32)
    conv1_b = np.asarray(inputs["conv1_b"], np.float32)
    pcaps_w = np.asarray(inputs["pcaps_w"], np.float32)
    pcaps_b = np.asarray(inputs["pcaps_b"], np.float32)
    route_w = np.asarray(inputs["route_w"], np.float32)

    # conv1 im2col patches + ones row (bias)
    xs = x[:, 0]  # [16,29,29,29]
    s0, s1, s2 = xs.strides[1:]
    patches = np.lib.stride_tricks.as_strided(
        xs, (B, 3, 3, 3, 27, 27, 27),
        (xs.strides[0], s0, s1, s2, s0, s1, s2))
    xp_full = np.empty((B, K1, S1), np.float32)
    xp_full[:, :27] = patches.reshape(B, 27, S1)
    xp_full[:, 27] = 1.0

    w1s = np.empty((K1, 128), np.float32)
    w1s[:27] = conv1_w.reshape(128, 27).T
    w1s[27] = conv1_b

    pcw = np.ascontiguousarray(
        pcaps_w.reshape(128, 128, OFF).transpose(2, 1, 0))
    if PC_NP is not None and PC_NP is not np.float32:
        pcw = pcw.astype(PC_NP)
    elif PC_NP is None:
        import ml_dtypes
        pcw = pcw.astype(ml_dtypes.bfloat16)
    pcb = pcaps_b.reshape(128, 1).copy()
    rwt = np.ascontiguousarray(route_w.reshape(C, 128, SLO * CAP * O))

    w1d = np.vstack([np.asarray(inputs["dec_w1"], np.float32),
                     np.asarray(inputs["dec_b1"], np.float32)[None]]).astype(np.float16)
    w2d = np.vstack([np.asarray(inputs["dec_w2"], np.float32),
                     np.asarray(inputs["dec_b2"], np.float32)[None]]).astype(np.float16)
    w3 = np.asarray(inputs["dec_w3"], np.float32).astype(np.float16)
    b3 = np.asarray(inputs["dec_b3"], np.float32).astype(np.float16)
    w4 = np.asarray(inputs["dec_w4"], np.float32).astype(np.float16)
    b4 = np.asarray(inputs["dec_b4"], np.float32).astype(np.float16)
    idt = np.eye(16, dtype=np.float32)

    in_maps = []
    for k in range(N_CORES):
        in_maps.append({
            "xp": np.ascontiguousarray(xp_full[2 * k:2 * k + 2]),
            "w1s": w1s, "pcw": pcw, "pcb": pcb, "rw": rwt,
            "w1d": w1d, "w2d": w2d,
            "w3d": np.ascontiguousarray(w3[:, k * D3PC:(k + 1) * D3PC]),
            "b3d": np.ascontiguousarray(b3[None, k * D3PC:(k + 1) * D3PC]),
            "w4d": np.ascontiguousarray(
                w4[:, k * D4PC:(k + 1) * D4PC]).reshape(KT4, 128, D4PC),
            "b4d": np.ascontiguousarray(b4[None, k * D4PC:(k + 1) * D4PC]),
            "idt": idt,
        })
    return in_maps


def run(inputs, stage="full", trace=False):
    key = stage
    if key not in _CACHE:
        _CACHE[key] = build(stage)
    nc = _CACHE[key]
    in_maps = _host_prep(inputs)
    res = run_bass_kernel_spmd(nc, in_maps, core_ids=list(range(N_CORES)),
                               trace=trace)
    return res


def kernel(**inputs):
    res = run(inputs, stage="full")
    classes = np.concatenate([res.results[k]["cls_out"] for k in range(N_CORES)], 0)
    recon = np.concatenate([res.results[k]["rec_out"] for k in range(N_CORES)], 1)
    return classes, recon
